# revision 17
# baseline (speedup 1.0000x reference)
"""Backward_projection (FBP: ramp filter + backprojection) on 8 trn2 NeuronCores.

Per core (data-parallel over batch x pixel-columns; core = (pq, bh) with
pq in 0..3 = 32-column image shard, bh in 0..1 = 128-batch shard):

  Phase 1 (filter): q^T = F^T x^T per angle via PE matmuls (fp16 in, fp32
    PSUM), writing HBM rows QT2[a*183 + d] = [q[d] for 128 batches | pad]
    in fp16 (512B rows).
  Phase 2 (backprojection as one big matmul): out[b, p] =
    sum_k q^T[k, b] * S[k, p] over k = (a, d) in 408 chunks of 128 rows.
    S is the static backprojection matrix (2 tent-interpolation weights per
    (angle, pixel) column), stored fp16 in HBM [408, 128, 4096] per pixel
    shard and streamed through the PE as the moving operand. q^T chunks are
    the stationaries, PSUM accumulates all 285 angles exactly.

The S shards (427MB each) are cached on-device across calls via a sharded
jax.device_put, so steady-state calls only transfer the sinogram x.
"""

import numpy as np
import sys

sys.path.insert(0, "/opt/trn_rl_repo")

# --- geometry constants (match the FBP reference) ---
N = 128
MIN_PT, MAX_PT = -20.0, 20.0
CELL = (MAX_PT - MIN_PT) / N
RHO = float(np.sqrt(2.0) * 20.0)
A = 285
D = 183
DC = 2.0 * RHO / D
PAD = 512
B = 256

NJ = 32            # image columns per core
NB = 128           # batches per core
NPIX = N * NJ      # 4096 pixels per core
ELEM = 128         # fp16 elements per QT2 row (q for 128 batches)
KROWS = A * D      # 52155 stacked (a, d) rows
NK = (KROWS + 127) // 128   # 408 contraction chunks
CH1 = 8            # phase-1 angles per chunk (one PSUM bank per angle)


def _filter_matrix():
    n = (np.fft.fftfreq(PAD) * PAD).astype(np.int64)
    h = np.zeros(PAD, np.float64)
    h[0] = 1.0 / (4.0 * DC * DC)
    odd = (n % 2) != 0
    h[odd] = -1.0 / (np.pi * n[odd] * DC) ** 2
    idx = (np.arange(D)[None, :] - np.arange(D)[:, None]) % PAD  # [d_in, d_out]
    return (h[idx] * (12.0 * DC * np.pi / A)).astype(np.float32)


def _backproj_tables():
    c = MIN_PT + (np.arange(N) + 0.5) * CELL
    X, Y = np.meshgrid(c, c, indexing="ij")
    th = (np.arange(A) + 0.5) * np.pi / A
    t = np.cos(th)[:, None] * X.ravel()[None, :] + np.sin(th)[:, None] * Y.ravel()[None, :]
    k = (t - (-RHO + 0.5 * DC)) / DC
    k0 = np.clip(np.floor(k), 0, D - 2).astype(np.int32)
    w = np.clip(k - k0, 0.0, 1.0).astype(np.float32)
    return k0.reshape(A, N, N), w.reshape(A, N, N)


_F32 = _filter_matrix()
_K0, _W = _backproj_tables()

# FC dram layout [2, 128, 184] fp16: F split into two d_in chunks
_FC = np.zeros((2, 128, 184), np.float16)
_FC[0, 0:128, 0:183] = _F32[0:128, :]
_FC[1, 0:55, 0:183] = _F32[128:183, :]


def _s_shard(pq):
    """S matrix shard [NK, 128, NPIX] fp16 for pixel columns pq*32..pq*32+31."""
    j0 = pq * NJ
    S = np.zeros((NK * 128, NPIX), np.float16)
    p = np.arange(NPIX)
    i = p // NJ
    jl = p % NJ
    for a in range(A):
        kk = _K0[a, i, j0 + jl]
        ww = _W[a, i, j0 + jl]
        rows = a * D + kk
        S[rows, p] = (1.0 - ww).astype(np.float16)
        S[rows + 1, p] = ww.astype(np.float16)
    return S.reshape(NK, 128, NPIX)


def build_nc(reps=1):
    from contextlib import ExitStack
    from concourse import bass, bacc, mybir

    f16 = mybir.dt.float16
    f32 = mybir.dt.float32

    ch1 = [(s, min(CH1, A - s)) for s in range(0, A, CH1)]
    C1 = [(r, s, n) for r in range(reps) for (s, n) in ch1]
    cumA1 = np.cumsum([c[2] for c in C1]).tolist()
    n1 = len(ch1)

    nc = bacc.Bacc("TRN2", target_bir_lowering=False, debug=False)

    x16 = nc.declare_dram_parameter("x16", [128, A, D], f16, isOutput=False)
    fc_h = nc.declare_dram_parameter("fc", [2, 128, 184], f16, isOutput=False)
    s_h = nc.declare_dram_parameter("sm", [NK, 128, NPIX], f16, isOutput=False)
    y_h = nc.declare_dram_parameter("y", [128, NPIX], f32, isOutput=True)

    qt2 = nc.dram_tensor("qt2", [KROWS + 69, ELEM], f16)   # flat (a*183+d) rows

    es = ExitStack()
    sb = lambda name, shape, dt: es.enter_context(nc.sbuf_tensor(name, shape, dt))
    XB = [sb(f"xb{i}", [128, CH1, 256], f16) for i in range(2)]
    XT = [sb(f"xt{i}", [128, CH1, 2, 128], f16) for i in range(2)]
    STA = [sb(f"sta{i}", [128, CH1, 128], f16) for i in range(2)]
    STB = [sb(f"stb{i}", [128, CH1, 128], f16) for i in range(2)]
    FC = sb("fcs", [128, 2, 184], f16)
    SLAB = [sb(f"slab{i}", [128, NPIX], f16) for i in range(2)]
    QTL = [sb(f"qtl{i}", [128, 128], f16) for i in range(2)]
    OUTS = sb("outs", [128, NPIX], f32)
    PS = es.enter_context(nc.psum_tensor("acc", [128, 8, 512], f32))

    sem_names = [
        "s_xb", "s_xt", "s_pe1", "s_qact", "s_qt2",
        "s_sl0", "s_sl1", "s_ql0", "s_ql1", "s_pe2", "s_outc", "s_yw", "s_fc", "s_ms",
    ]
    sems = {n: es.enter_context(nc.semaphore(n)) for n in sem_names}

    def kpar(k):
        return ("s_sl0" if k % 2 == 0 else "s_sl1", "s_ql0" if k % 2 == 0 else "s_ql1")

    with nc.Block() as block:

        @block.sync
        def _(sync: bass.BassEngine):
            sync.wait_ge(sems["s_ms"], 2)
            sync.dma_start(out=FC[:, :, :], in_=fc_h[:, :, :].transpose([1, 0, 2])).then_inc(sems["s_fc"], 16)
            for gi, (r, a0, cn) in enumerate(C1):
                ci = gi % n1
                if gi >= 2:
                    sync.wait_ge(sems["s_xt"], 16 * 2 * cumA1[gi - 2])   # XB[gi%2] free
                    sync.wait_ge(sems["s_pe1"], gi - 1)                  # XT[gi%2] free
                sync.dma_start(out=XB[gi % 2][:, 0:cn, 0:D], in_=x16[:, a0 : a0 + cn, :]).then_inc(sems["s_xb"], 16)
                sync.wait_ge(sems["s_xb"], 16 * (gi + 1))
                for ai in range(cn):
                    sync.dma_start_transpose(XT[gi % 2][:, ai, 0, :], XB[gi % 2][:, ai, 0:128]).then_inc(sems["s_xt"], 16)
                    sync.dma_start_transpose(XT[gi % 2][:, ai, 1, :], XB[gi % 2][:, ai, 128:256]).then_inc(sems["s_xt"], 16)
                # QT2 row writes for previous chunk (same rep)
                if ci >= 1:
                    _, p0, pn = C1[gi - 1]
                    if gi >= 2:
                        sync.wait_ge(sems["s_qt2"], 16 * 2 * cumA1[gi - 2])
                    sync.wait_ge(sems["s_qact"], cumA1[gi - 1])
                    for ai in range(pn):
                        a = p0 + ai
                        sync.dma_start(out=qt2[a * D : a * D + 128, :], in_=STA[(gi - 1) % 2][:, ai, :]).then_inc(sems["s_qt2"], 16)
                        sync.dma_start(out=qt2[a * D + 128 : a * D + 183, :], in_=STB[(gi - 1) % 2][0:55, ai, :]).then_inc(sems["s_qt2"], 16)
                if ci == n1 - 1:
                    # flush this rep's final chunk, then stream phase-2 S/qT chunks
                    sync.wait_ge(sems["s_qt2"], 16 * 2 * cumA1[gi - 1])
                    sync.wait_ge(sems["s_qact"], cumA1[gi])
                    for ai in range(cn):
                        a = a0 + ai
                        sync.dma_start(out=qt2[a * D : a * D + 128, :], in_=STA[gi % 2][:, ai, :]).then_inc(sems["s_qt2"], 16)
                        sync.dma_start(out=qt2[a * D + 128 : a * D + 183, :], in_=STB[gi % 2][0:55, ai, :]).then_inc(sems["s_qt2"], 16)
                    sync.wait_ge(sems["s_qt2"], 16 * 2 * A * (r + 1))
                    for k in range(NK):
                        slp, qlp = kpar(k)
                        gk = r * NK + k
                        ksz = min(128, KROWS - k * 128)
                        if k >= 2:
                            sync.wait_ge(sems[slp], 16 * (gk // 2))
                            sync.wait_ge(sems[qlp], 16 * (gk // 2))
                            sync.wait_ge(sems["s_pe2"], gk - 1)   # SLAB/QTL free
                        sync.dma_start(out=SLAB[k % 2][0:ksz, :], in_=s_h[k, 0:ksz, :]).then_inc(sems[slp], 16)
                        sync.dma_start(out=QTL[k % 2][0:ksz, :], in_=qt2[k * 128 : k * 128 + ksz, :]).then_inc(sems[qlp], 16)
            sync.wait_ge(sems["s_outc"], 8)
            sync.dma_start(out=y_h[:, :], in_=OUTS[:, :]).then_inc(sems["s_yw"], 16)
            sync.wait_ge(sems["s_yw"], 16)

        @block.tensor
        def _(tensor):
            tensor.wait_ge(sems["s_fc"], 16)
            for gi, (r, a0, cn) in enumerate(C1):
                ci = gi % n1
                tensor.wait_ge(sems["s_xt"], 16 * 2 * cumA1[gi])
                if gi >= 1:
                    tensor.wait_ge(sems["s_qact"], cumA1[gi - 1])   # PSUM banks free
                for mc in range(2):        # d_out chunk
                    msz = 128 if mc == 0 else 55
                    col = 256 * mc
                    for kc in range(2):    # d_in chunk
                        ksz = 128 if kc == 0 else 55
                        for ai in range(cn):
                            mm = tensor.matmul(
                                PS[0:msz, ai, col : col + 128],
                                FC[0:ksz, kc, 128 * mc : 128 * mc + msz],
                                XT[gi % 2][0:ksz, ai, kc, :],
                                start=(kc == 0),
                                stop=(kc == 1),
                                skip_group_check=True,
                            )
                mm.then_inc(sems["s_pe1"], 1)
                if ci == n1 - 1:
                    # phase-2: accumulate out[b, p] over 408 qT/S chunks
                    tensor.wait_ge(sems["s_qact"], cumA1[gi])
                    for k in range(NK):
                        slp, qlp = kpar(k)
                        gk = r * NK + k
                        ksz = min(128, KROWS - k * 128)
                        tensor.wait_ge(sems[slp], 16 * (gk // 2 + 1))
                        tensor.wait_ge(sems[qlp], 16 * (gk // 2 + 1))
                        for nt in range(8):
                            mm = tensor.matmul(
                                PS[:, nt, :],
                                QTL[k % 2][0:ksz, :],
                                SLAB[k % 2][0:ksz, 512 * nt : 512 * (nt + 1)],
                                start=(k == 0),
                                stop=(k == NK - 1),
                                skip_group_check=True,
                            )
                        mm.then_inc(sems["s_pe2"], 1)

        @block.scalar
        def _(scalar):
            for gi, (r, a0, cn) in enumerate(C1):
                scalar.wait_ge(sems["s_pe1"], gi + 1)
                if gi >= 2:
                    scalar.wait_ge(sems["s_qt2"], 16 * 2 * cumA1[gi - 2])   # ST buffers free
                for ai in range(cn):
                    scalar.mul(STA[gi % 2][:, ai, 0:128], PS[:, ai, 0:128], 1.0)
                    scalar.mul(STB[gi % 2][0:55, ai, 0:128], PS[0:55, ai, 256:384], 1.0).then_inc(sems["s_qact"], 1)

        @block.vector
        def _(vector):
            vector.memset(XB[0][:, :, :], 0.0).then_inc(sems["s_ms"], 1)
            vector.memset(XB[1][:, :, :], 0.0).then_inc(sems["s_ms"], 1)
            vector.wait_ge(sems["s_pe2"], reps * NK)
            for nb in range(8):
                vector.tensor_copy(OUTS[:, 512 * nb : 512 * (nb + 1)], PS[:, nb, :]).then_inc(sems["s_outc"], 1)

    nc.compile()
    es.close()
    return nc


_CACHE = {}


def _get_state(reps=1):
    if reps in _CACHE:
        return _CACHE[reps]
    import jax
    from jax.sharding import Mesh, PartitionSpec, NamedSharding
    from jax.experimental.shard_map import shard_map
    from concourse import bass2jax, mybir

    nc = build_nc(reps)
    bass2jax.install_neuronx_cc_hook()

    pname = nc.partition_id_tensor.name if nc.partition_id_tensor else None
    in_names, out_names, out_avals, zero_outs = [], [], [], []
    for alloc in nc.m.functions[0].allocations:
        if not isinstance(alloc, mybir.MemoryLocationSet):
            continue
        name = alloc.memorylocations[0].name
        if alloc.kind == "ExternalInput":
            if name != pname:
                in_names.append(name)
        elif alloc.kind == "ExternalOutput":
            out_names.append(name)
            shape = tuple(alloc.tensor_shape)
            dtype = mybir.dt.np(alloc.dtype)
            out_avals.append(jax.core.ShapedArray(shape, dtype))
            zero_outs.append(np.zeros(shape, dtype))
    n_params = len(in_names)
    all_names = in_names + out_names
    if pname is not None:
        all_names = all_names + [pname]

    def _body(*args):
        operands = list(args)
        if pname is not None:
            operands.append(bass2jax.partition_id_tensor())
        outs = bass2jax._bass_exec_p.bind(
            *operands,
            out_avals=tuple(out_avals),
            in_names=tuple(all_names),
            out_names=tuple(out_names),
            lowering_input_output_aliases=(),
            sim_require_finite=True,
            sim_require_nnan=True,
            nc=nc,
        )
        return tuple(outs)

    devices = jax.devices()[:8]
    mesh = Mesh(np.asarray(devices), ("core",))
    spec = PartitionSpec("core")
    donate = tuple(range(n_params, n_params + len(out_names)))
    fn = jax.jit(
        shard_map(_body, mesh=mesh, in_specs=(spec,) * (n_params + len(out_names)),
                  out_specs=(spec,) * len(out_names), check_rep=False),
        donate_argnums=donate, keep_unused=True,
    )
    sharding = NamedSharding(mesh, spec)

    # static per-core inputs, device-cached: fc (tiny) and sm (427MB/core)
    fc_dev = jax.device_put(np.concatenate([_FC] * 8, axis=0), sharding)
    shards = {}
    sm_parts = []
    for core in range(8):
        pq = core % 4
        if pq not in shards:
            shards[pq] = _s_shard(pq)
        sm_parts.append(shards[pq])
    sm_all = np.concatenate(sm_parts, axis=0)
    del shards, sm_parts
    sm_dev = jax.device_put(sm_all, sharding)
    del sm_all

    state = {
        "fn": fn, "in_names": in_names, "out_names": out_names,
        "fc": fc_dev, "sm": sm_dev,
        "zero_shapes": [(tuple(z.shape), z.dtype) for z in zero_outs],
        "sharding": sharding,
    }
    _CACHE[reps] = state
    return state


def kernel(x: np.ndarray, reps: int = 1) -> np.ndarray:
    import jax
    st = _get_state(reps)
    x = np.asarray(x, dtype=np.float32)
    x16_all = np.concatenate(
        [x[(c // 4) * NB : (c // 4 + 1) * NB].astype(np.float16) for c in range(8)], axis=0
    )
    args = []
    for name in st["in_names"]:
        if name == "x16":
            args.append(x16_all)
        elif name == "fc":
            args.append(st["fc"])
        elif name == "sm":
            args.append(st["sm"])
        else:
            raise KeyError(name)
    zeros = [
        jax.device_put(np.zeros(shape, dt), st["sharding"])
        for (shape, dt) in st["zero_shapes"]
    ]
    outs = st["fn"](*args, *zeros)
    y_all = np.asarray(outs[st["out_names"].index("y")])   # [8*128, NPIX]
    out = np.empty((B, N, N), np.float32)
    for core in range(8):
        pq, bh = core % 4, core // 4
        y = y_all[core * 128 : (core + 1) * 128].reshape(128, N, NJ)  # [b, i, jl]
        out[bh * NB : (bh + 1) * NB, :, pq * NJ : (pq + 1) * NJ] = y
    return out


if __name__ == "__main__":
    rng = np.random.default_rng(0)
    x = rng.standard_normal((B, A, D)).astype(np.float32)
    y = kernel(x)
    print(y.shape, y.dtype, float(np.abs(y).max()))
